# revision 1
# baseline (speedup 1.0000x reference)
"""Two-branch 2-layer GCN (EncoderGCN2) on 8 trn2 NeuronCores.

Strategy (graph/data parallel per the sharding hint):
  - Destination-node sharding: each of the 8 cores owns 1/8 of the
    destination nodes of BOTH graphs (x and y).
  - GCN normalization factorized: with dis = deg^-1/2,
        out = diag(dis) * (A+I) * diag(dis) * (X W) + b
    so each core builds the dis-scaled feature table H0 = diag(dis) X W1
    (replicated), fetches rows h0[src] for its edges with the batched
    dma_gather custom instruction (int16 indices over table halves),
    scatter-adds per 128-destination block with a one-hot selection
    matrix on the TensorEngine (PSUM accumulation), and applies the
    dis[dst] scale + bias (+ReLU) in the block epilogue.
  - The layer-1 epilogue directly computes H2pre = diag(dis) relu(out1) W2
    for the core's destination slice; an AllGather shares the full
    H2pre table with every core for layer 2's cross-partition gathers
    (the "halo exchange").
  - Phases are interleaved x/y so the AllGathers hide under the other
    graph's compute.

Tables / matmuls run in bf16 (fp32 PSUM accumulation, fp32 epilogues).
"""

import numpy as np
import ml_dtypes

import concourse.bass as bass
import concourse.bacc as bacc
import concourse.mybir as mybir
import concourse.tile as tile
from concourse.bass_utils import run_bass_kernel_spmd
from concourse.masks import make_identity

f32 = mybir.dt.float32
bf16 = mybir.dt.bfloat16
i16 = mybir.dt.int16
i32 = mybir.dt.int32

N = 50000
E = 800000
IN = 128
HID = 128
OUT = 64
NC = 8
SH = N // NC               # 6250 destination rows per core
NBLK = (SH + 127) // 128   # 49 blocks
SHP = NBLK * 128           # 6272 padded rows per core
NPAD = ((N + 127) // 128) * 128   # 50048
NTA = NPAD // 128          # 391 stage-A tiles
TBL2 = NC * SHP            # 50176 rows of the layer-2 table
HALF1 = NPAD // 2          # 25024 (H0 half)
HALF2 = TBL2 // 2          # 25088 (H2 half)

GCAP = 16                  # max tiles per dma_gather (2048 idxs)
SK = 8                     # S_T tiles per DVE build op
ACHUNK = 16                # stage-A tiles per XT load
DEBUG = False


def _layer_meta(es_by_core, blk_by_core, off_by_core, half_split):
    """Build per-layer tile schedule with edges split by table half.

    es_by_core[c]: per-core edge source row ids (already in the target
    table's index space), grouped contiguously by destination block
    (blk_by_core[c]) with in-block offsets off_by_core[c].
    """
    ntl = np.zeros((NC, NBLK), np.int64)
    nth = np.zeros((NC, NBLK), np.int64)
    parts = []
    for c in range(NC):
        es, b, o = es_by_core[c], blk_by_core[c], off_by_core[c]
        lo = es < half_split
        parts.append((es, b, o, lo))
        for blk in range(NBLK):
            m = b == blk
            nlo = int((lo & m).sum())
            nhi = int((~lo & m).sum())
            ntl[c, blk] = (nlo + 127) // 128
            nth[c, blk] = (nhi + 127) // 128
    NTL = ntl.max(axis=0)
    NTH = nth.max(axis=0)
    TT = int(NTL.sum() + NTH.sum())

    # tile schedule + gather groups (identical across cores)
    sched = []          # (block, first, last) per tile
    groups = []         # (tile_start, ntiles, half)
    tbase_lo = np.zeros(NBLK, np.int64)
    tbase_hi = np.zeros(NBLK, np.int64)
    t = 0
    for blk in range(NBLK):
        ntiles = int(NTL[blk] + NTH[blk])
        tbase_lo[blk] = t
        tbase_hi[blk] = t + int(NTL[blk])
        for i in range(ntiles):
            sched.append((blk, i == 0, i == ntiles - 1))
        for half, cnt in ((0, int(NTL[blk])), (1, int(NTH[blk]))):
            s = t if half == 0 else t + int(NTL[blk])
            while cnt > 0:
                n = min(cnt, GCAP)
                groups.append((s, n, half))
                s += n
                cnt -= n
        t += ntiles

    idx16 = np.zeros((NC, TT * 128), np.int16)
    idx32 = np.zeros((NC, TT * 128), np.int32)
    dstl = np.full((NC, TT * 128), 255.0, np.float32)
    for c in range(NC):
        es, b, o, lo = parts[c]
        for half, mask, base, halfoff in (
            (0, lo, tbase_lo, 0), (1, ~lo, tbase_hi, half_split)):
            eh, bh, oh = es[mask], b[mask], o[mask]
            cnt = np.bincount(bh, minlength=NBLK)
            starts = np.zeros(NBLK, np.int64)
            starts[1:] = np.cumsum(cnt)[:-1]
            pos = np.arange(len(eh)) - starts[bh]
            slots = base[bh] * 128 + pos
            idx16[c, slots] = (eh - halfoff).astype(np.int16)
            idx32[c, slots] = eh.astype(np.int32)
            dstl[c, slots] = oh
    # dma_gather wrapped layout: flat j -> [16k + j%16, j//16], replicated k=0..7
    wrap = idx16.reshape(NC, TT * 8, 16).transpose(0, 2, 1)       # [NC,16,TT*8]
    idx16w = np.tile(wrap, (1, 8, 1)).copy()                      # [NC,128,TT*8]
    idx32 = idx32.reshape(NC, TT, 128).transpose(0, 2, 1).copy()  # [NC,128,TT]
    dstl = dstl.reshape(NC, TT, 128).transpose(0, 2, 1).copy()    # [NC,128,TT]
    return dict(TT=TT, sched=sched, groups=groups, idx=idx16w, idx32=idx32,
                dstl=dstl)


def _graph_meta(edge_index):
    src = np.asarray(edge_index[0]).astype(np.int64)
    dst = np.asarray(edge_index[1]).astype(np.int64)
    deg = (np.bincount(dst, minlength=N) + 1).astype(np.float32)
    dis = (1.0 / np.sqrt(deg)).astype(np.float32)

    loops = np.arange(N, dtype=np.int64)
    src_all = np.concatenate([src, loops])
    dst_all = np.concatenate([dst, loops])
    core = dst_all // SH

    esA, esC, blks, offs = [], [], [], []
    disb = np.zeros((NC, 128, NBLK), np.float32)
    for c in range(NC):
        m = core == c
        es, ed = src_all[m], dst_all[m]
        l = ed - c * SH
        order = np.argsort(l, kind="stable")
        es, l = es[order], l[order]
        esA.append(es)
        esC.append((es // SH) * SHP + (es % SH))
        blks.append(l >> 7)
        offs.append(l & 127)
        lv = np.arange(SHP)
        gv = np.minimum(c * SH + lv, N - 1)
        disb[c] = np.where(lv < SH, dis[gv], 0.0).reshape(NBLK, 128).T

    l1 = _layer_meta(esA, blks, offs, HALF1)
    l2 = _layer_meta(esC, blks, offs, HALF2)

    dn = np.zeros(NPAD, np.float32)
    dn[:N] = dis
    disn = dn.reshape(NTA, 128).T.copy()   # [128, NTA]
    return dict(l1=l1, l2=l2, disb=disb, disn=disn, dis=dis)


def _build(meta):
    DTT = bf16

    nc = bacc.Bacc("TRN2", target_bir_lowering=False, debug=False, num_devices=NC)
    gs = ("x", "y")
    XT, W1, W2, B1, B2 = {}, {}, {}, {}, {}
    IDX1, IDX2, DSTL1, DSTL2, DISN, DISB = {}, {}, {}, {}, {}, {}
    IDX1B, IDX2B = {}, {}
    H0, H2loc, H2all, OUTT, DBG = {}, {}, {}, {}, {}
    for g in gs:
        T1 = meta[g]["l1"]["TT"]
        T2 = meta[g]["l2"]["TT"]
        XT[g] = nc.dram_tensor(f"xt_{g}", [128, NPAD], DTT, kind="ExternalInput")
        W1[g] = nc.dram_tensor(f"w1_{g}", [128, HID], DTT, kind="ExternalInput")
        W2[g] = nc.dram_tensor(f"w2_{g}", [128, OUT], DTT, kind="ExternalInput")
        B1[g] = nc.dram_tensor(f"b1_{g}", [128, HID], f32, kind="ExternalInput")
        B2[g] = nc.dram_tensor(f"b2_{g}", [128, OUT], f32, kind="ExternalInput")
        IDX1[g] = nc.dram_tensor(f"idx1_{g}", [128, T1 * 8], i16, kind="ExternalInput")
        IDX2[g] = nc.dram_tensor(f"idx2_{g}", [128, T2 * 8], i16, kind="ExternalInput")
        IDX1B[g] = nc.dram_tensor(f"idx1b_{g}", [128, T1], i32, kind="ExternalInput")
        IDX2B[g] = nc.dram_tensor(f"idx2b_{g}", [128, T2], i32, kind="ExternalInput")
        DSTL1[g] = nc.dram_tensor(f"dstl1_{g}", [128, T1], f32, kind="ExternalInput")
        DSTL2[g] = nc.dram_tensor(f"dstl2_{g}", [128, T2], f32, kind="ExternalInput")
        DISN[g] = nc.dram_tensor(f"disn_{g}", [128, NTA], f32, kind="ExternalInput")
        DISB[g] = nc.dram_tensor(f"disb_{g}", [128, NBLK], f32, kind="ExternalInput")
        H0[g] = nc.dram_tensor(f"h0_{g}", [NPAD, HID], DTT)
        H2loc[g] = nc.dram_tensor(f"h2loc_{g}", [SHP, 128], DTT)
        H2all[g] = nc.dram_tensor(f"h2all_{g}", [TBL2, 128], DTT)
        OUTT[g] = nc.dram_tensor(f"out_{g}", [SHP, OUT], f32, kind="ExternalOutput")
        if DEBUG:
            DBG[g] = nc.dram_tensor(f"dbg_{g}", [NPAD, HID], f32, kind="ExternalOutput")
    IOTA = nc.dram_tensor("iota", [128, 128], f32, kind="ExternalInput")

    with tile.TileContext(nc) as tc:
        with (
            tc.tile_pool(name="meta", bufs=1) as mp,
            tc.tile_pool(name="xt", bufs=2) as xp,
            tc.tile_pool(name="h0s", bufs=2) as h0p,
            tc.tile_pool(name="gat", bufs=6) as gp,
            tc.tile_pool(name="st", bufs=3) as stp,
            tc.tile_pool(name="epi", bufs=8) as ep,
            tc.tile_pool(name="stage", bufs=2) as sgp,
            tc.tile_pool(name="psA", bufs=2, space="PSUM") as ppa,
            tc.tile_pool(name="psE", bufs=4, space="PSUM") as ppe,
            tc.tile_pool(name="psX", bufs=2, space="PSUM") as ppx,
        ):
            iota_sb = mp.tile([128, 128], f32)
            nc.sync.dma_start(out=iota_sb[:], in_=IOTA[:, :])
            ident = mp.tile([128, 128], DTT)
            make_identity(nc, ident[:])
            w1_sb, w2_sb, b1_sb, b2_sb = {}, {}, {}, {}
            idx_sb, idx32_sb, dstl_sb, disn_sb, disb_sb = {}, {}, {}, {}, {}
            for g in gs:
                T1 = meta[g]["l1"]["TT"]
                T2 = meta[g]["l2"]["TT"]
                w1_sb[g] = mp.tile([128, HID], DTT, tag=f"w1{g}", name=f"w1sb_{g}")
                nc.sync.dma_start(out=w1_sb[g][:], in_=W1[g][:, :])
                w2_sb[g] = mp.tile([128, OUT], DTT, tag=f"w2{g}", name=f"w2sb_{g}")
                nc.sync.dma_start(out=w2_sb[g][:], in_=W2[g][:, :])
                b1_sb[g] = mp.tile([128, HID], f32, tag=f"b1{g}", name=f"b1sb_{g}")
                nc.sync.dma_start(out=b1_sb[g][:], in_=B1[g][:, :])
                b2_sb[g] = mp.tile([128, OUT], f32, tag=f"b2{g}", name=f"b2sb_{g}")
                nc.sync.dma_start(out=b2_sb[g][:], in_=B2[g][:, :])
                idx_sb[g, 1] = mp.tile([128, T1 * 8], i16, tag=f"i1{g}", name=f"idx1sb_{g}")
                nc.sync.dma_start(out=idx_sb[g, 1][:], in_=IDX1[g][:, :])
                idx_sb[g, 2] = mp.tile([128, T2 * 8], i16, tag=f"i2{g}", name=f"idx2sb_{g}")
                nc.sync.dma_start(out=idx_sb[g, 2][:], in_=IDX2[g][:, :])
                idx32_sb[g, 1] = mp.tile([128, T1], i32, tag=f"j1{g}", name=f"idx1bsb_{g}")
                nc.sync.dma_start(out=idx32_sb[g, 1][:], in_=IDX1B[g][:, :])
                idx32_sb[g, 2] = mp.tile([128, T2], i32, tag=f"j2{g}", name=f"idx2bsb_{g}")
                nc.sync.dma_start(out=idx32_sb[g, 2][:], in_=IDX2B[g][:, :])
                dstl_sb[g, 1] = mp.tile([128, T1], f32, tag=f"d1{g}", name=f"dstl1sb_{g}")
                nc.sync.dma_start(out=dstl_sb[g, 1][:], in_=DSTL1[g][:, :])
                dstl_sb[g, 2] = mp.tile([128, T2], f32, tag=f"d2{g}", name=f"dstl2sb_{g}")
                nc.sync.dma_start(out=dstl_sb[g, 2][:], in_=DSTL2[g][:, :])
                disn_sb[g] = mp.tile([128, NTA], f32, tag=f"dn{g}", name=f"disnsb_{g}")
                nc.sync.dma_start(out=disn_sb[g][:], in_=DISN[g][:, :])
                disb_sb[g] = mp.tile([128, NBLK], f32, tag=f"db{g}", name=f"disbsb_{g}")
                nc.sync.dma_start(out=disb_sb[g][:], in_=DISB[g][:, :])

            def stage_A(g):
                h0v = H0[g].ap().rearrange("(nb p) c -> nb p c", p=128)
                for t0 in range(0, NTA, ACHUNK):
                    csz = min(ACHUNK, NTA - t0)
                    xtc = xp.tile([128, csz * 128], DTT, tag="xtc")
                    nc.sync.dma_start(
                        out=xtc[:], in_=XT[g][:, t0 * 128:(t0 + csz) * 128])
                    h0s = h0p.tile([128, csz * 128], DTT, tag="h0s")
                    for s0 in range(0, csz, 4):
                        ssz = min(4, csz - s0)
                        ps = ppa.tile([128, ssz * 128], f32, space="PSUM", tag="psA")
                        for i in range(ssz):
                            nc.tensor.matmul(
                                out=ps[:, i * 128:(i + 1) * 128],
                                lhsT=xtc[:, (s0 + i) * 128:(s0 + i + 1) * 128],
                                rhs=w1_sb[g][:],
                                start=True, stop=True,
                            )
                        nc.vector.tensor_tensor(
                            out=h0s[:, s0 * 128:(s0 + ssz) * 128]
                                .rearrange("p (t c) -> p t c", c=128),
                            in0=ps[:].rearrange("p (t c) -> p t c", c=128),
                            in1=disn_sb[g][:, t0 + s0:t0 + s0 + ssz]
                                .rearrange("p (t c) -> p t c", c=1)
                                .to_broadcast([128, ssz, 128]),
                            op=mybir.AluOpType.mult,
                        )
                    nc.sync.dma_start(
                        out=h0v[t0:t0 + csz].rearrange("nb p c -> p nb c"),
                        in_=h0s[:].rearrange("p (nb c) -> p nb c", c=128),
                    )

            def edge_stage(g, layer):
                m = meta[g]["l1" if layer == 1 else "l2"]
                TT, sched, groups = m["TT"], m["sched"], m["groups"]
                D = HID if layer == 1 else OUT
                GW = 128                       # gathered row width (cols)
                if layer == 1:
                    halves = (H0[g][0:HALF1, :], H0[g][HALF1:2 * HALF1, :])
                else:
                    halves = (H2all[g][0:HALF2, :], H2all[g][HALF2:2 * HALF2, :])
                isb = idx_sb[g, layer]
                jsb = idx32_sb[g, layer]
                dsb = dstl_sb[g, layer]

                # tile index -> (gather tile handle, col offset)
                gt_of = [None] * TT
                gidx = 0
                ps = None
                stb = None
                s0 = 0
                stage_t = None
                nstage = 0

                def flush(bb, nst, st_t):
                    lo = bb - nst + 1
                    if layer == 1:
                        dst = H2loc[g].ap().rearrange("(nb p) c -> nb p c", p=128)
                        w = 128
                    else:
                        dst = OUTT[g].ap().rearrange("(nb p) c -> nb p c", p=128)
                        w = OUT
                    nc.sync.dma_start(
                        out=dst[lo:lo + nst].rearrange("nb p c -> p nb c"),
                        in_=st_t[:, :nst * w].rearrange("p (nb c) -> p nb c", c=w),
                    )

                for j in range(TT):
                    if gidx < len(groups) and groups[gidx][0] == j:
                        ts, ntl, half = groups[gidx]
                        gt = gp.tile([128, ntl * GW], DTT, tag="gt")
                        if True:  # dma_gather outperforms the indirect path
                            nc.gpsimd.dma_gather(
                                out_ap=gt[:].rearrange("p (t c) -> p t c", c=GW),
                                in_ap=halves[half],
                                idxs_ap=isb[:, ts * 8:(ts + ntl) * 8],
                                num_idxs=ntl * 128,
                                num_idxs_reg=ntl * 128,
                                elem_size=GW,
                                single_packet=False,
                            )
                        else:
                            table_full = (H0[g] if layer == 1 else H2all[g])
                            for k in range(ntl):
                                nc.gpsimd.indirect_dma_start(
                                    out=gt[:, k * GW:(k + 1) * GW],
                                    out_offset=None,
                                    in_=table_full.ap(),
                                    in_offset=bass.IndirectOffsetOnAxis(
                                        ap=jsb[:, ts + k:ts + k + 1], axis=0),
                                )
                        for k in range(ntl):
                            gt_of[ts + k] = (gt, k * GW)
                        gidx += 1
                    if j % SK == 0:
                        ssz = min(SK, TT - j)
                        stb = stp.tile([128, ssz * 128], DTT, tag="stb")
                        nc.vector.tensor_tensor(
                            out=stb[:].rearrange("p (t c) -> p t c", c=128),
                            in0=dsb[:, j:j + ssz]
                                .rearrange("p (t c) -> p t c", c=1)
                                .to_broadcast([128, ssz, 128]),
                            in1=iota_sb[:].rearrange("p (t c) -> p t c", t=1)
                                .to_broadcast([128, ssz, 128]),
                            op=mybir.AluOpType.is_equal,
                        )
                        s0 = j
                    b, first, last = sched[j]
                    if first:
                        ps = ppe.tile([128, D], f32, space="PSUM", tag="psE")
                    gtile, goff = gt_of[j]
                    nc.tensor.matmul(
                        out=ps[:],
                        lhsT=stb[:, (j - s0) * 128:(j - s0 + 1) * 128],
                        rhs=gtile[:, goff:goff + D],
                        start=first, stop=last,
                    )
                    if last:
                        if stage_t is None:
                            stage_t = sgp.tile(
                                [128, 8 * (128 if layer == 1 else OUT)],
                                DTT if layer == 1 else f32,
                                tag=f"stage{layer}", name=f"staget_{g}{layer}")
                        dcol = disb_sb[g][:, b:b + 1]
                        if layer == 1:
                            z1 = ep.tile([128, HID], f32, tag="z1")
                            nc.vector.tensor_scalar(
                                out=z1[:], in0=ps[:], scalar1=dcol,
                                scalar2=None, op0=mybir.AluOpType.mult)
                            z2 = ep.tile([128, HID], f32, tag="z2")
                            nc.vector.tensor_tensor(
                                out=z2[:], in0=z1[:], in1=b1_sb[g][:],
                                op=mybir.AluOpType.add)
                            r = ep.tile([128, HID], DTT, tag="r")
                            nc.scalar.activation(
                                out=r[:], in_=z2[:],
                                func=mybir.ActivationFunctionType.Relu)
                            pst = ppx.tile([128, HID], DTT, space="PSUM", tag="psX")
                            nc.tensor.transpose(
                                out=pst[:], in_=r[:], identity=ident[:])
                            rt = ep.tile([128, HID], DTT, tag="rt")
                            nc.vector.tensor_copy(out=rt[:], in_=pst[:])
                            ph2 = ppx.tile([128, OUT], f32, space="PSUM", tag="psX")
                            nc.tensor.matmul(
                                out=ph2[:], lhsT=rt[:], rhs=w2_sb[g][:],
                                start=True, stop=True)
                            nc.vector.tensor_scalar(
                                out=stage_t[:, (b % 8) * 128:(b % 8) * 128 + OUT],
                                in0=ph2[:], scalar1=dcol,
                                scalar2=None, op0=mybir.AluOpType.mult)
                        else:
                            o1 = ep.tile([128, OUT], f32, tag="o1")
                            nc.vector.tensor_scalar(
                                out=o1[:], in0=ps[:], scalar1=dcol,
                                scalar2=None, op0=mybir.AluOpType.mult)
                            nc.vector.tensor_tensor(
                                out=stage_t[:, (b % 8) * OUT:(b % 8 + 1) * OUT],
                                in0=o1[:], in1=b2_sb[g][:],
                                op=mybir.AluOpType.add)
                        nstage += 1
                        if b % 8 == 7 or b == NBLK - 1:
                            flush(b, nstage, stage_t)
                            stage_t = None
                            nstage = 0

            def allgather(g):
                nc.gpsimd.collective_compute(
                    "AllGather",
                    mybir.AluOpType.bypass,
                    replica_groups=[list(range(NC))],
                    ins=[H2loc[g].ap().opt()],
                    outs=[H2all[g].ap().opt()],
                )

            stage_A("x")
            stage_A("y")
            edge_stage("x", 1)
            allgather("x")
            edge_stage("y", 1)
            allgather("y")
            edge_stage("x", 2)
            edge_stage("y", 2)

            if DEBUG:
                for g in gs:
                    for t0 in range(0, NTA, 16):
                        csz = min(16, NTA - t0)
                        dv = h0p.tile([128, csz * 128], f32, tag="dbg")
                        nc.sync.dma_start(
                            out=dv[:].rearrange("p (nb c) -> p nb c", c=128),
                            in_=H0[g].ap().rearrange("(nb p) c -> nb p c", p=128)
                                [t0:t0 + csz].rearrange("nb p c -> p nb c"))
                        nc.sync.dma_start(
                            out=DBG[g].ap().rearrange("(nb p) c -> nb p c", p=128)
                                [t0:t0 + csz].rearrange("nb p c -> p nb c"),
                            in_=dv[:].rearrange("p (nb c) -> p nb c", c=128))

    nc.compile()
    return nc


def _in_maps(meta, inputs):
    np_t = ml_dtypes.bfloat16
    iota = np.broadcast_to(np.arange(128, dtype=np.float32), (128, 128)).copy()
    shared = {"iota": iota}
    weights = {
        "x": (inputs["W1x"], inputs["b1x"], inputs["W2x"], inputs["b2x"],
              inputs["x_data_matrix"]),
        "y": (inputs["W1y"], inputs["b1y"], inputs["W2y"], inputs["b2y"],
              inputs["y_data_matrix"]),
    }
    for g in ("x", "y"):
        w1, b1, w2, b2, xd = weights[g]
        xt = np.zeros((128, NPAD), np.float32)
        xt[:, :N] = np.asarray(xd, np.float32).T
        shared[f"xt_{g}"] = xt.astype(np_t)
        shared[f"w1_{g}"] = np.asarray(w1, np.float32).astype(np_t)
        shared[f"w2_{g}"] = np.asarray(w2, np.float32).astype(np_t)
        shared[f"b1_{g}"] = np.broadcast_to(
            np.asarray(b1, np.float32), (128, HID)).copy()
        shared[f"b2_{g}"] = np.broadcast_to(
            np.asarray(b2, np.float32), (128, OUT)).copy()
        shared[f"disn_{g}"] = meta[g]["disn"]
    maps = []
    for c in range(NC):
        m = dict(shared)
        for g in ("x", "y"):
            mg = meta[g]
            m[f"idx1_{g}"] = mg["l1"]["idx"][c]
            m[f"idx2_{g}"] = mg["l2"]["idx"][c]
            m[f"idx1b_{g}"] = mg["l1"]["idx32"][c]
            m[f"idx2b_{g}"] = mg["l2"]["idx32"][c]
            m[f"dstl1_{g}"] = mg["l1"]["dstl"][c]
            m[f"dstl2_{g}"] = mg["l2"]["dstl"][c]
            m[f"disb_{g}"] = mg["disb"][c]
        maps.append(m)
    return maps


def run(inputs, trace=False):
    meta = {
        "x": _graph_meta(inputs["x_edge_index"]),
        "y": _graph_meta(inputs["y_edge_index"]),
    }
    nc = _build(meta)
    maps = _in_maps(meta, inputs)
    kwargs = {}
    if trace:
        kwargs = dict(trace=True, trace_cores=[0])
    res = run_bass_kernel_spmd(nc, maps, core_ids=list(range(NC)), **kwargs)
    outs = {}
    for g in ("x", "y"):
        full = np.empty((N, OUT), np.float32)
        for c in range(NC):
            full[c * SH:(c + 1) * SH] = res.results[c][f"out_{g}"][:SH]
        outs[g] = full
    return (outs["x"], outs["y"]), res


def kernel(**inputs):
    (ox, oy), _ = run(inputs)
    return ox, oy



# revision 17
# speedup vs baseline: 1.2720x; 1.2720x over previous
"""Two-branch 2-layer GCN (EncoderGCN2) on 8 trn2 NeuronCores — v2.

Destination-node sharding (each core owns 1/8 of dst rows of both
graphs).  Changes vs v1:
  - Self-loop edges removed from the gather streams.  Layer-1 self
    contribution is a per-block matmul from XSELF (per-core input =
    this core's XT column slice) accumulated directly into the block
    PSUM; layer-2 self contribution is the ph2 block saved in SBUF
    during the layer-1 epilogue.
  - dis[src] folded into XT host-side; dis[dst] applied via
    per-partition scalars in the epilogues (incl. pre-scaling the relu
    output before the W2 matmul so the layer-2 table needs no scale).
  - Gather groups span dst blocks within 8-block super-groups: fewer,
    larger dma_gather instructions.
  - AllGather of the layer-2 table is chunked (3 chunks per graph,
    fired as each chunk's blocks flush) into a Shared DRAM tensor.
  - stage_A(y) interleaved into the l1(x) gather emission.
"""

import numpy as np
import ml_dtypes

import concourse.bass as bass
import concourse.bacc as bacc
import concourse.mybir as mybir
import concourse.tile as tile
from concourse.bass_utils import run_bass_kernel_spmd
from concourse.masks import make_identity

f32 = mybir.dt.float32
bf16 = mybir.dt.bfloat16
i16 = mybir.dt.int16

N = 50000
E = 800000
IN = 128
HID = 128
OUT = 64
NC = 8
SH = N // NC               # 6250 destination rows per core
NBLK = (SH + 127) // 128   # 49 blocks
SHP = NBLK * 128           # 6272 padded rows per core
NPAD = ((N + 127) // 128) * 128   # 50048
NTA = NPAD // 128          # 391 stage-A tiles
HALF1 = NPAD // 2          # 25024
TBL2 = NC * SHP            # 50176
HALF2 = TBL2 // 2          # 25088

SG = 4                     # blocks per super-group (one PSUM bank per open block)
SGS = [list(range(s, min(s + SG, NBLK))) for s in range(0, NBLK, SG)]
# allgather chunks = ranges of super-groups -> contiguous block ranges
CHUNK_SGS = [(0, 4), (4, 8), (8, len(SGS))]
CHUNK_BLKS = [sum((SGS[i] for i in range(a, b)), []) for a, b in CHUNK_SGS]
CHUNK_FIRST = [blks[0] for blks in CHUNK_BLKS]
CHUNK_ROWS = [len(blks) * 128 for blks in CHUNK_BLKS]
CHUNK_BASE = np.concatenate([[0], np.cumsum([NC * r for r in CHUNK_ROWS])[:-1]])
CHUNK_OF_BLK = np.zeros(NBLK, np.int64)
for k, blks in enumerate(CHUNK_BLKS):
    CHUNK_OF_BLK[np.array(blks)] = k

GSZ_TILES = 16             # tiles per dma_gather (2048 idxs)
SK = 8                     # S_T tiles per DVE build op
ACHUNK = 16                # stage-A tiles per XT load
NQ = 1                     # swdge queues used round-robin
DEBUG = False              # expose H2loc/H2all as outputs


def _layer_meta(es_by_core, blk_by_core, off_by_core, half_split):
    """Tile schedule with per-super-group half runs.

    Tile order: per super-group, all its blocks' lo-half tiles, then all
    hi-half tiles.  Gather groups are contiguous tile runs within one
    (super-group, half) run, chunked at GSZ_TILES.
    """
    ntl = np.zeros((NC, NBLK), np.int64)
    nth = np.zeros((NC, NBLK), np.int64)
    parts = []
    for c in range(NC):
        es, b, o = es_by_core[c], blk_by_core[c], off_by_core[c]
        lo = es < half_split
        parts.append((es, b, o, lo))
        for blk in range(NBLK):
            m = b == blk
            ntl[c, blk] = (int((lo & m).sum()) + 127) // 128
            nth[c, blk] = (int((~lo & m).sum()) + 127) // 128
    NTL = ntl.max(axis=0)
    NTH = nth.max(axis=0)
    TT = int(NTL.sum() + NTH.sum())

    tbase_lo = np.zeros(NBLK, np.int64)
    tbase_hi = np.zeros(NBLK, np.int64)
    tile_blk = []               # block of each tile
    groups = []                 # (tile_start, ntiles, half)
    sg_last_tile = []           # last tile index of each super-group
    t = 0
    for sg in SGS:
        for half in (0, 1):
            cnt_arr = NTL if half == 0 else NTH
            base_arr = tbase_lo if half == 0 else tbase_hi
            run_start = t
            for blk in sg:
                base_arr[blk] = t
                tile_blk.extend([blk] * int(cnt_arr[blk]))
                t += int(cnt_arr[blk])
            n = t - run_start
            s = run_start
            if n > 0:
                k = (n + GSZ_TILES - 1) // GSZ_TILES
                base, rem = divmod(n, k)
                for i in range(k):
                    gn = base + (1 if i < rem else 0)
                    groups.append((s, gn, half))
                    s += gn
        sg_last_tile.append(t - 1)

    first_t = np.full(NBLK, -1, np.int64)
    last_t = np.full(NBLK, -1, np.int64)
    for ti, blk in enumerate(tile_blk):
        if first_t[blk] < 0:
            first_t[blk] = ti
        last_t[blk] = ti
    sched = [(blk, ti == first_t[blk], ti == last_t[blk])
             for ti, blk in enumerate(tile_blk)]
    # blocks with zero tiles (no edges at all): epilogue-only
    zero_blocks = [b for b in range(NBLK) if first_t[b] < 0]

    idx16 = np.zeros((NC, max(TT, 1) * 128), np.int16)
    dstl = np.full((NC, max(TT, 1) * 128), 255.0, np.float32)
    for c in range(NC):
        es, b, o, lo = parts[c]
        for half, mask, base, halfoff in (
                (0, lo, tbase_lo, 0), (1, ~lo, tbase_hi, half_split)):
            eh, bh, oh = es[mask], b[mask], o[mask]
            cnt = np.bincount(bh, minlength=NBLK)
            starts = np.zeros(NBLK, np.int64)
            starts[1:] = np.cumsum(cnt)[:-1]
            pos = np.arange(len(eh)) - starts[bh]
            slots = base[bh] * 128 + pos
            idx16[c, slots] = (eh - halfoff).astype(np.int16)
            dstl[c, slots] = oh
    wrap = idx16.reshape(NC, -1, 16).transpose(0, 2, 1)
    idx16w = np.tile(wrap, (1, 8, 1)).copy()                      # [NC,128,TT*8]
    dstl = dstl.reshape(NC, -1, 128).transpose(0, 2, 1).copy()    # [NC,128,TT]
    return dict(TT=TT, sched=sched, groups=groups, idx=idx16w, dstl=dstl,
                sg_last_tile=sg_last_tile, zero_blocks=zero_blocks)


def _graph_meta(edge_index):
    src = np.asarray(edge_index[0]).astype(np.int64)
    dst = np.asarray(edge_index[1]).astype(np.int64)
    deg = (np.bincount(dst, minlength=N) + 1).astype(np.float32)
    dis = (1.0 / np.sqrt(deg)).astype(np.float32)

    core = dst // SH
    esA, esC, blks, offs = [], [], [], []
    disb = np.zeros((NC, 128, NBLK), np.float32)
    for c in range(NC):
        m = core == c
        es, ed = src[m], dst[m]
        l = ed - c * SH
        order = np.argsort(l * (N + 1) + es, kind="stable")
        es, l = es[order], l[order]
        esA.append(es)
        sc, sl = es // SH, es % SH
        sb, so = sl >> 7, sl & 127
        k = CHUNK_OF_BLK[sb]
        esC.append(CHUNK_BASE[k] + sc * np.asarray(CHUNK_ROWS)[k]
                   + (sb - np.asarray(CHUNK_FIRST)[k]) * 128 + so)
        blks.append(l >> 7)
        offs.append(l & 127)
        lv = np.arange(SHP)
        gv = np.minimum(c * SH + lv, N - 1)
        disb[c] = np.where(lv < SH, dis[gv], 0.0).reshape(NBLK, 128).T

    l1 = _layer_meta(esA, blks, offs, HALF1)
    l2 = _layer_meta(esC, blks, offs, HALF2)
    return dict(l1=l1, l2=l2, disb=disb, dis=dis)


def _build(meta):
    DTT = bf16
    nc = bacc.Bacc("TRN2", target_bir_lowering=False, debug=False,
                   num_devices=NC, num_swdge_queues=max(NQ, 1))
    gs = ("x", "y")
    XT, XSELF, W1, W2, B1, B2 = {}, {}, {}, {}, {}, {}
    IDX1, IDX2, DSTL1, DSTL2, DISB = {}, {}, {}, {}, {}
    H0, H2loc, H2all, OUTT, DBGA = {}, {}, {}, {}, {}
    for g in gs:
        T1 = meta[g]["l1"]["TT"]
        T2 = meta[g]["l2"]["TT"]
        XT[g] = nc.dram_tensor(f"xt_{g}", [128, NPAD], DTT, kind="ExternalInput")
        XSELF[g] = nc.dram_tensor(f"xself_{g}", [128, SHP], DTT,
                                  kind="ExternalInput")
        W1[g] = nc.dram_tensor(f"w1_{g}", [128, HID], DTT, kind="ExternalInput")
        W2[g] = nc.dram_tensor(f"w2_{g}", [128, OUT], DTT, kind="ExternalInput")
        B1[g] = nc.dram_tensor(f"b1_{g}", [128, HID], f32, kind="ExternalInput")
        B2[g] = nc.dram_tensor(f"b2_{g}", [128, OUT], f32, kind="ExternalInput")
        IDX1[g] = nc.dram_tensor(f"idx1_{g}", [128, T1 * 8], i16, kind="ExternalInput")
        IDX2[g] = nc.dram_tensor(f"idx2_{g}", [128, T2 * 8], i16, kind="ExternalInput")
        DSTL1[g] = nc.dram_tensor(f"dstl1_{g}", [128, T1], f32, kind="ExternalInput")
        DSTL2[g] = nc.dram_tensor(f"dstl2_{g}", [128, T2], f32, kind="ExternalInput")
        DISB[g] = nc.dram_tensor(f"disb_{g}", [128, NBLK], f32, kind="ExternalInput")
        if DEBUG:
            H0[g] = nc.dram_tensor(f"h0_{g}", [NPAD, HID], DTT,
                                   kind="ExternalOutput")
            H2loc[g] = nc.dram_tensor(f"h2loc_{g}", [SHP, 128], DTT)
            H2all[g] = nc.dram_tensor(f"h2all_{g}", [TBL2, 128], DTT)
            DBGA[g] = nc.dram_tensor(f"dbga_{g}", [TBL2, 128], DTT,
                                     kind="ExternalOutput")
        else:
            H0[g] = nc.dram_tensor(f"h0_{g}", [NPAD, HID], DTT)
            H2loc[g] = nc.dram_tensor(f"h2loc_{g}", [SHP, 128], DTT)
            H2all[g] = nc.dram_tensor(f"h2all_{g}", [TBL2, 128], DTT,
                                      addr_space="Shared")
        OUTT[g] = nc.dram_tensor(f"out_{g}", [SHP, OUT], f32, kind="ExternalOutput")
    IOTA = nc.dram_tensor("iota", [128, 128], f32, kind="ExternalInput")

    with tile.TileContext(nc) as tc:
        with (
            tc.tile_pool(name="meta", bufs=1) as mp,
            tc.tile_pool(name="passm", bufs=2) as pmp,
            tc.tile_pool(name="xt", bufs=2) as xp,
            tc.tile_pool(name="h0s", bufs=2) as h0p,
            tc.tile_pool(name="gat", bufs=5) as gp,
            tc.tile_pool(name="st", bufs=3) as stp,
            tc.tile_pool(name="epi", bufs=8) as ep,
            tc.tile_pool(name="stage", bufs=2) as sgp,
            tc.tile_pool(name="psA", bufs=2, space="PSUM") as ppa,
            tc.tile_pool(name="psE", bufs=4, space="PSUM") as ppe,
            tc.tile_pool(name="psX", bufs=2, space="PSUM") as ppx,
        ):
            iota_sb = mp.tile([128, 128], f32)
            nc.sync.dma_start(out=iota_sb[:], in_=IOTA[:, :])
            ident = mp.tile([128, 128], DTT)
            make_identity(nc, ident[:])
            w1_sb, w2_sb, b1_sb, b2_sb, disb_sb = {}, {}, {}, {}, {}
            xself_sb, self2_sb = {}, {}
            for g in gs:
                w1_sb[g] = mp.tile([128, HID], DTT, tag=f"w1{g}", name=f"w1sb_{g}")
                nc.sync.dma_start(out=w1_sb[g][:], in_=W1[g][:, :])
                w2_sb[g] = mp.tile([128, OUT], DTT, tag=f"w2{g}", name=f"w2sb_{g}")
                nc.sync.dma_start(out=w2_sb[g][:], in_=W2[g][:, :])
                b1_sb[g] = mp.tile([128, HID], f32, tag=f"b1{g}", name=f"b1sb_{g}")
                nc.sync.dma_start(out=b1_sb[g][:], in_=B1[g][:, :])
                b2_sb[g] = mp.tile([128, OUT], f32, tag=f"b2{g}", name=f"b2sb_{g}")
                nc.sync.dma_start(out=b2_sb[g][:], in_=B2[g][:, :])
                disb_sb[g] = mp.tile([128, NBLK], f32, tag=f"db{g}",
                                     name=f"disbsb_{g}")
                nc.sync.dma_start(out=disb_sb[g][:], in_=DISB[g][:, :])
                xself_sb[g] = mp.tile([128, SHP], DTT, tag=f"xs{g}",
                                      name=f"xself_{g}")
                nc.sync.dma_start(out=xself_sb[g][:], in_=XSELF[g][:, :])
                self2_sb[g] = mp.tile([128, NBLK * OUT], DTT, tag=f"s2{g}",
                                      name=f"self2_{g}")

            def stage_A_chunks(g):
                h0v = H0[g].ap().rearrange("(nb p) c -> nb p c", p=128)
                for t0 in range(0, NTA, ACHUNK):
                    csz = min(ACHUNK, NTA - t0)
                    xtc = xp.tile([128, ACHUNK * 128], DTT, tag="xtc")
                    nc.sync.dma_start(
                        out=xtc[:, :csz * 128],
                        in_=XT[g][:, t0 * 128:(t0 + csz) * 128])
                    h0s = h0p.tile([128, ACHUNK * 128], DTT, tag="h0s")
                    for s0 in range(0, csz, 4):
                        ssz = min(4, csz - s0)
                        ps = ppa.tile([128, 4 * 128], f32, space="PSUM", tag="psA")
                        for i in range(ssz):
                            nc.tensor.matmul(
                                out=ps[:, i * 128:(i + 1) * 128],
                                lhsT=xtc[:, (s0 + i) * 128:(s0 + i + 1) * 128],
                                rhs=w1_sb[g][:],
                                start=True, stop=True,
                            )
                        nc.vector.tensor_copy(
                            out=h0s[:, s0 * 128:(s0 + ssz) * 128],
                            in_=ps[:, :ssz * 128])
                    nc.sync.dma_start(
                        out=h0v[t0:t0 + csz].rearrange("nb p c -> p nb c"),
                        in_=h0s[:, :csz * 128]
                            .rearrange("p (nb c) -> p nb c", c=128),
                    )
                    yield

            def ag_chunk(g, k):
                r0 = CHUNK_FIRST[k] * 128
                rows = CHUNK_ROWS[k]
                nc.gpsimd.collective_compute(
                    "AllGather",
                    mybir.AluOpType.bypass,
                    replica_groups=[list(range(NC))],
                    ins=[H2loc[g].ap()[r0:r0 + rows, :]],
                    outs=[H2all[g].ap()[int(CHUNK_BASE[k]):
                                        int(CHUNK_BASE[k]) + NC * rows, :]],
                )

            def edge_pass(g, layer, interleave=None, chunk_cb=None):
                m = meta[g]["l1" if layer == 1 else "l2"]
                TT, sched, groups = m["TT"], m["sched"], m["groups"]
                sg_last = {t: i for i, t in enumerate(m["sg_last_tile"])}
                D = HID if layer == 1 else OUT
                if layer == 1:
                    halves = (H0[g][0:HALF1, :], H0[g][HALF1:2 * HALF1, :])
                else:
                    halves = (H2all[g][0:HALF2, :], H2all[g][HALF2:2 * HALF2, :])
                isb = pmp.tile([128, TT * 8], i16, tag=f"idx{layer}",
                               name=f"isb_{g}{layer}")
                nc.sync.dma_start(out=isb[:],
                                  in_=(IDX1[g] if layer == 1 else IDX2[g])[:, :])
                dsb = pmp.tile([128, TT], f32, tag=f"dstl{layer}",
                               name=f"dsb_{g}{layer}")
                nc.sync.dma_start(out=dsb[:],
                                  in_=(DSTL1[g] if layer == 1 else DSTL2[g])[:, :])

                ps_of = {}
                gt_of = [None] * TT
                stb_of = [None] * TT
                state = dict(stage_t=None, nstage=0, first_blk=0)
                gq = 0

                def flush():
                    if layer == 1:
                        dst = H2loc[g].ap().rearrange("(nb p) c -> nb p c", p=128)
                        w = 128
                    else:
                        dst = OUTT[g].ap().rearrange("(nb p) c -> nb p c", p=128)
                        w = OUT
                    nst = state["nstage"]
                    nc.sync.dma_start(
                        out=dst[state["first_blk"]:state["first_blk"] + nst]
                            .rearrange("nb p c -> p nb c"),
                        in_=state["stage_t"][:, :nst * w]
                            .rearrange("p (nb c) -> p nb c", c=w),
                    )
                    state["stage_t"] = None
                    state["nstage"] = 0

                def open_block(b):
                    # one PSUM bank per open accumulator (accumulation groups
                    # own their bank; slicing a shared bank corrupts partials)
                    pst_ = ppe.tile([128, D], f32, space="PSUM", tag="psE",
                                    name="psE")
                    ps = pst_[:]
                    if layer == 1:
                        # self-loop contribution: H0[own dst rows of block b]
                        nc.tensor.matmul(
                            out=ps,
                            lhsT=xself_sb[g][:, b * 128:(b + 1) * 128],
                            rhs=w1_sb[g][:],
                            start=True, stop=False,
                        )
                    else:
                        nc.tensor.matmul(
                            out=ps,
                            lhsT=ident[:],
                            rhs=self2_sb[g][:, b * OUT:(b + 1) * OUT],
                            start=True, stop=False,
                        )
                    ps_of[b] = ps
                    return ps

                def epilogue(b, ps):
                    dcol = disb_sb[g][:, b:b + 1]
                    if state["stage_t"] is None:
                        state["stage_t"] = sgp.tile(
                            [128, SG * (128 if layer == 1 else OUT)],
                            DTT if layer == 1 else f32,
                            tag=f"stage{layer}", name=f"staget_{g}{layer}")
                        state["first_blk"] = b - (b % SG)
                    if layer == 1:
                        z2 = ep.tile([128, HID], f32, tag="z2")
                        nc.vector.tensor_scalar(
                            out=z2[:], in0=ps, scalar1=dcol,
                            scalar2=None, op0=mybir.AluOpType.mult)
                        z3 = ep.tile([128, HID], f32, tag="z3")
                        nc.vector.tensor_tensor(
                            out=z3[:], in0=z2[:], in1=b1_sb[g][:],
                            op=mybir.AluOpType.add)
                        r = ep.tile([128, HID], f32, tag="r")
                        nc.scalar.activation(
                            out=r[:], in_=z3[:],
                            func=mybir.ActivationFunctionType.Relu)
                        rp = ep.tile([128, HID], DTT, tag="rp")
                        nc.vector.tensor_scalar(
                            out=rp[:], in0=r[:], scalar1=dcol,
                            scalar2=None, op0=mybir.AluOpType.mult)
                        pst = ppx.tile([128, HID], DTT, space="PSUM", tag="psX")
                        nc.tensor.transpose(out=pst[:], in_=rp[:],
                                            identity=ident[:])
                        rt = ep.tile([128, HID], DTT, tag="rt")
                        nc.vector.tensor_copy(out=rt[:], in_=pst[:])
                        ph2 = ppx.tile([128, OUT], f32, space="PSUM", tag="psX")
                        nc.tensor.matmul(out=ph2[:], lhsT=rt[:], rhs=w2_sb[g][:],
                                         start=True, stop=True)
                        nc.vector.tensor_copy(
                            out=state["stage_t"][:, (b % SG) * 128:
                                                 (b % SG) * 128 + OUT],
                            in_=ph2[:])
                        nc.vector.tensor_copy(
                            out=self2_sb[g][:, b * OUT:(b + 1) * OUT],
                            in_=ph2[:])
                    else:
                        z2 = ep.tile([128, OUT], f32, tag="z2")
                        nc.vector.tensor_scalar(
                            out=z2[:], in0=ps, scalar1=dcol,
                            scalar2=None, op0=mybir.AluOpType.mult)
                        nc.vector.tensor_tensor(
                            out=state["stage_t"][:, (b % SG) * OUT:
                                                 (b % SG + 1) * OUT],
                            in0=z2[:], in1=b2_sb[g][:],
                            op=mybir.AluOpType.add)
                    state["nstage"] += 1
                    if state["nstage"] == SG or b == NBLK - 1:
                        flush()

                # zero-edge blocks are impossible at this edge density
                assert not m["zero_blocks"], m["zero_blocks"]

                for (ts, ntl, half) in groups:
                    gt = gp.tile([128, GSZ_TILES * 128], DTT, tag="gt")
                    nc.gpsimd.dma_gather(
                        out_ap=gt[:, :ntl * 128]
                            .rearrange("p (t c) -> p t c", c=128),
                        in_ap=halves[half],
                        idxs_ap=isb[:, ts * 8:(ts + ntl) * 8],
                        num_idxs=ntl * 128,
                        num_idxs_reg=ntl * 128,
                        elem_size=128,
                        single_packet=False,
                        queue_num=gq % NQ,
                    )
                    gq += 1
                    for k in range(ntl):
                        gt_of[ts + k] = (gt, k * 128)
                    for s0 in range(ts, ts + ntl, SK):
                        ssz = min(SK, ts + ntl - s0)
                        stb = stp.tile([128, SK * 128], DTT, tag="stb")
                        nc.vector.tensor_tensor(
                            out=stb[:, :ssz * 128]
                                .rearrange("p (t c) -> p t c", c=128),
                            in0=dsb[:, s0:s0 + ssz]
                                .rearrange("p (t c) -> p t c", c=1)
                                .to_broadcast([128, ssz, 128]),
                            in1=iota_sb[:].rearrange("p (t c) -> p t c", t=1)
                                .to_broadcast([128, ssz, 128]),
                            op=mybir.AluOpType.is_equal,
                        )
                        for k in range(ssz):
                            stb_of[s0 + k] = (stb, k * 128)
                    for j in range(ts, ts + ntl):
                        b, first, last = sched[j]
                        if first:
                            open_block(b)
                        gtile, goff = gt_of[j]
                        stile, soff = stb_of[j]
                        nc.tensor.matmul(
                            out=ps_of[b],
                            lhsT=stile[:, soff:soff + 128],
                            rhs=gtile[:, goff:goff + D],
                            start=False, stop=last,
                        )
                        if last:
                            epilogue(b, ps_of.pop(b))
                        if j in sg_last and chunk_cb is not None:
                            chunk_cb(sg_last[j] + 1)
                    if interleave is not None:
                        next(interleave, None)

            def chunk_fire(g):
                fired = set()

                def cb(sgs_done):
                    for k, (a, b) in enumerate(CHUNK_SGS):
                        if sgs_done >= b and k not in fired:
                            fired.add(k)
                            ag_chunk(g, k)
                return cb

            ax = stage_A_chunks("x")
            for _ in ax:
                pass
            ay = stage_A_chunks("y")
            edge_pass("x", 1, interleave=ay, chunk_cb=chunk_fire("x"))
            for _ in ay:
                pass
            edge_pass("y", 1, chunk_cb=chunk_fire("y"))
            edge_pass("x", 2)
            edge_pass("y", 2)

            if DEBUG:
                for g in gs:
                    h2v = H2all[g].ap().rearrange("(nb p) c -> nb p c", p=128)
                    dbv = DBGA[g].ap().rearrange("(nb p) c -> nb p c", p=128)
                    for t0 in range(0, TBL2 // 128, 16):
                        csz = min(16, TBL2 // 128 - t0)
                        dv = h0p.tile([128, 16 * 128], DTT, tag="dbg")
                        nc.sync.dma_start(
                            out=dv[:, :csz * 128]
                                .rearrange("p (nb c) -> p nb c", c=128),
                            in_=h2v[t0:t0 + csz].rearrange("nb p c -> p nb c"))
                        nc.sync.dma_start(
                            out=dbv[t0:t0 + csz].rearrange("nb p c -> p nb c"),
                            in_=dv[:, :csz * 128]
                                .rearrange("p (nb c) -> p nb c", c=128))

    nc.compile()
    return nc


def _in_maps(meta, inputs):
    np_t = ml_dtypes.bfloat16
    iota = np.broadcast_to(np.arange(128, dtype=np.float32), (128, 128)).copy()
    shared = {"iota": iota}
    weights = {
        "x": (inputs["W1x"], inputs["b1x"], inputs["W2x"], inputs["b2x"],
              inputs["x_data_matrix"]),
        "y": (inputs["W1y"], inputs["b1y"], inputs["W2y"], inputs["b2y"],
              inputs["y_data_matrix"]),
    }
    xts = {}
    for g in ("x", "y"):
        w1, b1, w2, b2, xd = weights[g]
        dis = meta[g]["dis"]
        xt = np.zeros((128, NPAD), np.float32)
        xt[:, :N] = (np.asarray(xd, np.float32) * dis[:, None]).T
        xts[g] = xt.astype(np_t)
        shared[f"xt_{g}"] = xts[g]
        shared[f"w1_{g}"] = np.asarray(w1, np.float32).astype(np_t)
        shared[f"w2_{g}"] = np.asarray(w2, np.float32).astype(np_t)
        shared[f"b1_{g}"] = np.broadcast_to(
            np.asarray(b1, np.float32), (128, HID)).copy()
        shared[f"b2_{g}"] = np.broadcast_to(
            np.asarray(b2, np.float32), (128, OUT)).copy()
    maps = []
    for c in range(NC):
        m = dict(shared)
        for g in ("x", "y"):
            mg = meta[g]
            m[f"idx1_{g}"] = mg["l1"]["idx"][c]
            m[f"idx2_{g}"] = mg["l2"]["idx"][c]
            m[f"dstl1_{g}"] = mg["l1"]["dstl"][c]
            m[f"dstl2_{g}"] = mg["l2"]["dstl"][c]
            m[f"disb_{g}"] = mg["disb"][c]
            xs = np.zeros((128, SHP), np.float32)
            lo = c * SH
            hi = min(lo + SHP, NPAD)
            xs[:, :hi - lo] = xts[g][:, lo:hi].astype(np.float32)
            xs[:, SH:] = 0.0
            m[f"xself_{g}"] = xs.astype(np_t)
        maps.append(m)
    return maps


def run(inputs, trace=False):
    meta = {
        "x": _graph_meta(inputs["x_edge_index"]),
        "y": _graph_meta(inputs["y_edge_index"]),
    }
    nc = _build(meta)
    maps = _in_maps(meta, inputs)
    kwargs = {}
    if trace:
        kwargs = dict(trace=True, trace_cores=[0])
    res = run_bass_kernel_spmd(nc, maps, core_ids=list(range(NC)), **kwargs)
    outs = {}
    for g in ("x", "y"):
        full = np.empty((N, OUT), np.float32)
        for c in range(NC):
            full[c * SH:(c + 1) * SH] = res.results[c][f"out_{g}"][:SH]
        outs[g] = full
    return (outs["x"], outs["y"]), res


def kernel(**inputs):
    (ox, oy), _ = run(inputs)
    return ox, oy


# revision 18
# speedup vs baseline: 7.1012x; 5.5828x over previous
"""Two-branch 2-layer GCN (EncoderGCN2) on 8 trn2 NeuronCores — v2.

Destination-node sharding (each core owns 1/8 of dst rows of both
graphs).  Changes vs v1:
  - Self-loop edges removed from the gather streams.  Layer-1 self
    contribution is a per-block matmul from XSELF (per-core input =
    this core's XT column slice) accumulated directly into the block
    PSUM; layer-2 self contribution is the ph2 block saved in SBUF
    during the layer-1 epilogue.
  - dis[src] folded into XT host-side; dis[dst] applied via
    per-partition scalars in the epilogues (incl. pre-scaling the relu
    output before the W2 matmul so the layer-2 table needs no scale).
  - Gather groups span dst blocks within 8-block super-groups: fewer,
    larger dma_gather instructions.
  - AllGather of the layer-2 table is chunked (3 chunks per graph,
    fired as each chunk's blocks flush) into a Shared DRAM tensor.
  - stage_A(y) interleaved into the l1(x) gather emission.
"""

import numpy as np
import ml_dtypes

import concourse.bass as bass
import concourse.bacc as bacc
import concourse.mybir as mybir
import concourse.tile as tile
from concourse.bass_utils import run_bass_kernel_spmd
from concourse.masks import make_identity

f32 = mybir.dt.float32
bf16 = mybir.dt.bfloat16
i16 = mybir.dt.int16

N = 50000
E = 800000
IN = 128
HID = 128
OUT = 64
NC = 8
SH = N // NC               # 6250 destination rows per core
NBLK = (SH + 127) // 128   # 49 blocks
SHP = NBLK * 128           # 6272 padded rows per core
NPAD = ((N + 127) // 128) * 128   # 50048
NTA = NPAD // 128          # 391 stage-A tiles
HALF1 = NPAD // 2          # 25024
TBL2 = NC * SHP            # 50176
HALF2 = TBL2 // 2          # 25088

SG = 4                     # blocks per super-group (one PSUM bank per open block)
SGS = [list(range(s, min(s + SG, NBLK))) for s in range(0, NBLK, SG)]
# allgather chunks = ranges of super-groups -> contiguous block ranges
CHUNK_SGS = [(0, len(SGS))]
CHUNK_BLKS = [sum((SGS[i] for i in range(a, b)), []) for a, b in CHUNK_SGS]
CHUNK_FIRST = [blks[0] for blks in CHUNK_BLKS]
CHUNK_ROWS = [len(blks) * 128 for blks in CHUNK_BLKS]
CHUNK_BASE = np.concatenate([[0], np.cumsum([NC * r for r in CHUNK_ROWS])[:-1]])
CHUNK_OF_BLK = np.zeros(NBLK, np.int64)
for k, blks in enumerate(CHUNK_BLKS):
    CHUNK_OF_BLK[np.array(blks)] = k

GSZ_TILES = 16             # tiles per dma_gather (2048 idxs)
SK = 8                     # S_T tiles per DVE build op
ACHUNK = 16                # stage-A tiles per XT load
NQ = 1                     # swdge queues used round-robin
DEBUG = False              # expose H2loc/H2all as outputs


def _layer_meta(es_by_core, blk_by_core, off_by_core, half_split):
    """Tile schedule with per-super-group half runs.

    Tile order: per super-group, all its blocks' lo-half tiles, then all
    hi-half tiles.  Gather groups are contiguous tile runs within one
    (super-group, half) run, chunked at GSZ_TILES.
    """
    ntl = np.zeros((NC, NBLK), np.int64)
    nth = np.zeros((NC, NBLK), np.int64)
    parts = []
    for c in range(NC):
        es, b, o = es_by_core[c], blk_by_core[c], off_by_core[c]
        lo = es < half_split
        parts.append((es, b, o, lo))
        for blk in range(NBLK):
            m = b == blk
            ntl[c, blk] = (int((lo & m).sum()) + 127) // 128
            nth[c, blk] = (int((~lo & m).sum()) + 127) // 128
    NTL = ntl.max(axis=0)
    NTH = nth.max(axis=0)
    TT = int(NTL.sum() + NTH.sum())

    tbase_lo = np.zeros(NBLK, np.int64)
    tbase_hi = np.zeros(NBLK, np.int64)
    tile_blk = []               # block of each tile
    groups = []                 # (tile_start, ntiles, half)
    sg_last_tile = []           # last tile index of each super-group
    t = 0
    for sg in SGS:
        for half in (0, 1):
            cnt_arr = NTL if half == 0 else NTH
            base_arr = tbase_lo if half == 0 else tbase_hi
            run_start = t
            for blk in sg:
                base_arr[blk] = t
                tile_blk.extend([blk] * int(cnt_arr[blk]))
                t += int(cnt_arr[blk])
            n = t - run_start
            s = run_start
            if n > 0:
                k = (n + GSZ_TILES - 1) // GSZ_TILES
                base, rem = divmod(n, k)
                for i in range(k):
                    gn = base + (1 if i < rem else 0)
                    groups.append((s, gn, half))
                    s += gn
        sg_last_tile.append(t - 1)

    first_t = np.full(NBLK, -1, np.int64)
    last_t = np.full(NBLK, -1, np.int64)
    for ti, blk in enumerate(tile_blk):
        if first_t[blk] < 0:
            first_t[blk] = ti
        last_t[blk] = ti
    sched = [(blk, ti == first_t[blk], ti == last_t[blk])
             for ti, blk in enumerate(tile_blk)]
    # blocks with zero tiles (no edges at all): epilogue-only
    zero_blocks = [b for b in range(NBLK) if first_t[b] < 0]

    idx16 = np.zeros((NC, max(TT, 1) * 128), np.int16)
    dstl = np.full((NC, max(TT, 1) * 128), 255.0, np.float32)
    for c in range(NC):
        es, b, o, lo = parts[c]
        for half, mask, base, halfoff in (
                (0, lo, tbase_lo, 0), (1, ~lo, tbase_hi, half_split)):
            eh, bh, oh = es[mask], b[mask], o[mask]
            cnt = np.bincount(bh, minlength=NBLK)
            starts = np.zeros(NBLK, np.int64)
            starts[1:] = np.cumsum(cnt)[:-1]
            pos = np.arange(len(eh)) - starts[bh]
            slots = base[bh] * 128 + pos
            idx16[c, slots] = (eh - halfoff).astype(np.int16)
            dstl[c, slots] = oh
    wrap = idx16.reshape(NC, -1, 16).transpose(0, 2, 1)
    idx16w = np.tile(wrap, (1, 8, 1)).copy()                      # [NC,128,TT*8]
    dstl = dstl.reshape(NC, -1, 128).transpose(0, 2, 1).copy()    # [NC,128,TT]
    return dict(TT=TT, sched=sched, groups=groups, idx=idx16w, dstl=dstl,
                sg_last_tile=sg_last_tile, zero_blocks=zero_blocks)


def _graph_meta(edge_index):
    src = np.asarray(edge_index[0]).astype(np.int64)
    dst = np.asarray(edge_index[1]).astype(np.int64)
    deg = (np.bincount(dst, minlength=N) + 1).astype(np.float32)
    dis = (1.0 / np.sqrt(deg)).astype(np.float32)

    core = dst // SH
    esA, esC, blks, offs = [], [], [], []
    disb = np.zeros((NC, 128, NBLK), np.float32)
    for c in range(NC):
        m = core == c
        es, ed = src[m], dst[m]
        l = ed - c * SH
        order = np.argsort(l * (N + 1) + es, kind="stable")
        es, l = es[order], l[order]
        esA.append(es)
        sc, sl = es // SH, es % SH
        sb, so = sl >> 7, sl & 127
        k = CHUNK_OF_BLK[sb]
        esC.append(CHUNK_BASE[k] + sc * np.asarray(CHUNK_ROWS)[k]
                   + (sb - np.asarray(CHUNK_FIRST)[k]) * 128 + so)
        blks.append(l >> 7)
        offs.append(l & 127)
        lv = np.arange(SHP)
        gv = np.minimum(c * SH + lv, N - 1)
        disb[c] = np.where(lv < SH, dis[gv], 0.0).reshape(NBLK, 128).T

    l1 = _layer_meta(esA, blks, offs, HALF1)
    l2 = _layer_meta(esC, blks, offs, HALF2)
    return dict(l1=l1, l2=l2, disb=disb, dis=dis)


def _build(meta):
    DTT = bf16
    nc = bacc.Bacc("TRN2", target_bir_lowering=False, debug=False,
                   num_devices=NC, num_swdge_queues=max(NQ, 1),
                   dynamic_dma_scratch_size=32768)
    gs = ("x", "y")
    XT, XSELF, W1, W2, B1, B2 = {}, {}, {}, {}, {}, {}
    IDX1, IDX2, DSTL1, DSTL2, DISB = {}, {}, {}, {}, {}
    H0, H2loc, H2all, OUTT, DBGA = {}, {}, {}, {}, {}
    for g in gs:
        T1 = meta[g]["l1"]["TT"]
        T2 = meta[g]["l2"]["TT"]
        XT[g] = nc.dram_tensor(f"xt_{g}", [128, NPAD], DTT, kind="ExternalInput")
        XSELF[g] = nc.dram_tensor(f"xself_{g}", [128, SHP], DTT,
                                  kind="ExternalInput")
        W1[g] = nc.dram_tensor(f"w1_{g}", [128, HID], DTT, kind="ExternalInput")
        W2[g] = nc.dram_tensor(f"w2_{g}", [128, OUT], DTT, kind="ExternalInput")
        B1[g] = nc.dram_tensor(f"b1_{g}", [128, HID], f32, kind="ExternalInput")
        B2[g] = nc.dram_tensor(f"b2_{g}", [128, OUT], f32, kind="ExternalInput")
        IDX1[g] = nc.dram_tensor(f"idx1_{g}", [128, T1 * 8], i16, kind="ExternalInput")
        IDX2[g] = nc.dram_tensor(f"idx2_{g}", [128, T2 * 8], i16, kind="ExternalInput")
        DSTL1[g] = nc.dram_tensor(f"dstl1_{g}", [128, T1], f32, kind="ExternalInput")
        DSTL2[g] = nc.dram_tensor(f"dstl2_{g}", [128, T2], f32, kind="ExternalInput")
        DISB[g] = nc.dram_tensor(f"disb_{g}", [128, NBLK], f32, kind="ExternalInput")
        if DEBUG:
            H0[g] = nc.dram_tensor(f"h0_{g}", [NPAD, HID], DTT,
                                   kind="ExternalOutput")
            H2loc[g] = nc.dram_tensor(f"h2loc_{g}", [SHP, 128], DTT)
            H2all[g] = nc.dram_tensor(f"h2all_{g}", [TBL2, 128], DTT)
            DBGA[g] = nc.dram_tensor(f"dbga_{g}", [TBL2, 128], DTT,
                                     kind="ExternalOutput")
        else:
            H0[g] = nc.dram_tensor(f"h0_{g}", [NPAD, HID], DTT)
            H2loc[g] = nc.dram_tensor(f"h2loc_{g}", [SHP, 128], DTT)
            H2all[g] = nc.dram_tensor(f"h2all_{g}", [TBL2, 128], DTT,
                                      addr_space="Shared")
        OUTT[g] = nc.dram_tensor(f"out_{g}", [SHP, OUT], f32, kind="ExternalOutput")
    IOTA = nc.dram_tensor("iota", [128, 128], f32, kind="ExternalInput")

    with tile.TileContext(nc) as tc:
        with (
            tc.tile_pool(name="meta", bufs=1) as mp,
            tc.tile_pool(name="passm", bufs=2) as pmp,
            tc.tile_pool(name="xt", bufs=2) as xp,
            tc.tile_pool(name="h0s", bufs=2) as h0p,
            tc.tile_pool(name="gat", bufs=5) as gp,
            tc.tile_pool(name="st", bufs=3) as stp,
            tc.tile_pool(name="epi", bufs=8) as ep,
            tc.tile_pool(name="stage", bufs=2) as sgp,
            tc.tile_pool(name="psA", bufs=2, space="PSUM") as ppa,
            tc.tile_pool(name="psE", bufs=4, space="PSUM") as ppe,
            tc.tile_pool(name="psX", bufs=2, space="PSUM") as ppx,
        ):
            iota_sb = mp.tile([128, 128], f32)
            nc.sync.dma_start(out=iota_sb[:], in_=IOTA[:, :])
            ident = mp.tile([128, 128], DTT)
            make_identity(nc, ident[:])
            w1_sb, w2_sb, b1_sb, b2_sb, disb_sb = {}, {}, {}, {}, {}
            xself_sb, self2_sb = {}, {}
            for g in gs:
                w1_sb[g] = mp.tile([128, HID], DTT, tag=f"w1{g}", name=f"w1sb_{g}")
                nc.sync.dma_start(out=w1_sb[g][:], in_=W1[g][:, :])
                w2_sb[g] = mp.tile([128, OUT], DTT, tag=f"w2{g}", name=f"w2sb_{g}")
                nc.sync.dma_start(out=w2_sb[g][:], in_=W2[g][:, :])
                b1_sb[g] = mp.tile([128, HID], f32, tag=f"b1{g}", name=f"b1sb_{g}")
                nc.sync.dma_start(out=b1_sb[g][:], in_=B1[g][:, :])
                b2_sb[g] = mp.tile([128, OUT], f32, tag=f"b2{g}", name=f"b2sb_{g}")
                nc.sync.dma_start(out=b2_sb[g][:], in_=B2[g][:, :])
                disb_sb[g] = mp.tile([128, NBLK], f32, tag=f"db{g}",
                                     name=f"disbsb_{g}")
                nc.sync.dma_start(out=disb_sb[g][:], in_=DISB[g][:, :])
                xself_sb[g] = mp.tile([128, SHP], DTT, tag=f"xs{g}",
                                      name=f"xself_{g}")
                nc.sync.dma_start(out=xself_sb[g][:], in_=XSELF[g][:, :])
                self2_sb[g] = mp.tile([128, NBLK * OUT], DTT, tag=f"s2{g}",
                                      name=f"self2_{g}")

            def stage_A_chunks(g):
                h0v = H0[g].ap().rearrange("(nb p) c -> nb p c", p=128)
                for t0 in range(0, NTA, ACHUNK):
                    csz = min(ACHUNK, NTA - t0)
                    xtc = xp.tile([128, ACHUNK * 128], DTT, tag="xtc")
                    nc.sync.dma_start(
                        out=xtc[:, :csz * 128],
                        in_=XT[g][:, t0 * 128:(t0 + csz) * 128])
                    h0s = h0p.tile([128, ACHUNK * 128], DTT, tag="h0s")
                    for s0 in range(0, csz, 4):
                        ssz = min(4, csz - s0)
                        ps = ppa.tile([128, 4 * 128], f32, space="PSUM", tag="psA")
                        for i in range(ssz):
                            nc.tensor.matmul(
                                out=ps[:, i * 128:(i + 1) * 128],
                                lhsT=xtc[:, (s0 + i) * 128:(s0 + i + 1) * 128],
                                rhs=w1_sb[g][:],
                                start=True, stop=True,
                            )
                        nc.vector.tensor_copy(
                            out=h0s[:, s0 * 128:(s0 + ssz) * 128],
                            in_=ps[:, :ssz * 128])
                    nc.sync.dma_start(
                        out=h0v[t0:t0 + csz].rearrange("nb p c -> p nb c"),
                        in_=h0s[:, :csz * 128]
                            .rearrange("p (nb c) -> p nb c", c=128),
                    )
                    yield

            def ag_chunk(g, k):
                r0 = CHUNK_FIRST[k] * 128
                rows = CHUNK_ROWS[k]
                nc.gpsimd.collective_compute(
                    "AllGather",
                    mybir.AluOpType.bypass,
                    replica_groups=[list(range(NC))],
                    ins=[H2loc[g].ap()[r0:r0 + rows, :]],
                    outs=[H2all[g].ap()[int(CHUNK_BASE[k]):
                                        int(CHUNK_BASE[k]) + NC * rows, :]],
                )

            def edge_pass(g, layer, interleave=None, chunk_cb=None):
                m = meta[g]["l1" if layer == 1 else "l2"]
                TT, sched, groups = m["TT"], m["sched"], m["groups"]
                sg_last = {t: i for i, t in enumerate(m["sg_last_tile"])}
                D = HID if layer == 1 else OUT
                if layer == 1:
                    halves = (H0[g][0:HALF1, :], H0[g][HALF1:2 * HALF1, :])
                else:
                    halves = (H2all[g][0:HALF2, :], H2all[g][HALF2:2 * HALF2, :])
                isb = pmp.tile([128, TT * 8], i16, tag=f"idx{layer}",
                               name=f"isb_{g}{layer}")
                nc.sync.dma_start(out=isb[:],
                                  in_=(IDX1[g] if layer == 1 else IDX2[g])[:, :])
                dsb = pmp.tile([128, TT], f32, tag=f"dstl{layer}",
                               name=f"dsb_{g}{layer}")
                nc.sync.dma_start(out=dsb[:],
                                  in_=(DSTL1[g] if layer == 1 else DSTL2[g])[:, :])

                ps_of = {}
                gt_of = [None] * TT
                stb_of = [None] * TT
                state = dict(stage_t=None, nstage=0, first_blk=0)
                gq = 0

                def flush():
                    if layer == 1:
                        dst = H2loc[g].ap().rearrange("(nb p) c -> nb p c", p=128)
                        w = 128
                    else:
                        dst = OUTT[g].ap().rearrange("(nb p) c -> nb p c", p=128)
                        w = OUT
                    nst = state["nstage"]
                    nc.sync.dma_start(
                        out=dst[state["first_blk"]:state["first_blk"] + nst]
                            .rearrange("nb p c -> p nb c"),
                        in_=state["stage_t"][:, :nst * w]
                            .rearrange("p (nb c) -> p nb c", c=w),
                    )
                    state["stage_t"] = None
                    state["nstage"] = 0

                def open_block(b):
                    # one PSUM bank per open accumulator (accumulation groups
                    # own their bank; slicing a shared bank corrupts partials)
                    pst_ = ppe.tile([128, D], f32, space="PSUM", tag="psE",
                                    name="psE")
                    ps = pst_[:]
                    if layer == 1:
                        # self-loop contribution: H0[own dst rows of block b]
                        nc.tensor.matmul(
                            out=ps,
                            lhsT=xself_sb[g][:, b * 128:(b + 1) * 128],
                            rhs=w1_sb[g][:],
                            start=True, stop=False,
                        )
                    else:
                        nc.tensor.matmul(
                            out=ps,
                            lhsT=ident[:],
                            rhs=self2_sb[g][:, b * OUT:(b + 1) * OUT],
                            start=True, stop=False,
                        )
                    ps_of[b] = ps
                    return ps

                def epilogue(b, ps):
                    dcol = disb_sb[g][:, b:b + 1]
                    if state["stage_t"] is None:
                        state["stage_t"] = sgp.tile(
                            [128, SG * (128 if layer == 1 else OUT)],
                            DTT if layer == 1 else f32,
                            tag=f"stage{layer}", name=f"staget_{g}{layer}")
                        state["first_blk"] = b - (b % SG)
                    if layer == 1:
                        z2 = ep.tile([128, HID], f32, tag="z2")
                        nc.vector.tensor_scalar(
                            out=z2[:], in0=ps, scalar1=dcol,
                            scalar2=None, op0=mybir.AluOpType.mult)
                        z3 = ep.tile([128, HID], f32, tag="z3")
                        nc.vector.tensor_tensor(
                            out=z3[:], in0=z2[:], in1=b1_sb[g][:],
                            op=mybir.AluOpType.add)
                        r = ep.tile([128, HID], f32, tag="r")
                        nc.scalar.activation(
                            out=r[:], in_=z3[:],
                            func=mybir.ActivationFunctionType.Relu)
                        rp = ep.tile([128, HID], DTT, tag="rp")
                        nc.vector.tensor_scalar(
                            out=rp[:], in0=r[:], scalar1=dcol,
                            scalar2=None, op0=mybir.AluOpType.mult)
                        pst = ppx.tile([128, HID], DTT, space="PSUM", tag="psX")
                        nc.tensor.transpose(out=pst[:], in_=rp[:],
                                            identity=ident[:])
                        rt = ep.tile([128, HID], DTT, tag="rt")
                        nc.vector.tensor_copy(out=rt[:], in_=pst[:])
                        ph2 = ppx.tile([128, OUT], f32, space="PSUM", tag="psX")
                        nc.tensor.matmul(out=ph2[:], lhsT=rt[:], rhs=w2_sb[g][:],
                                         start=True, stop=True)
                        nc.vector.tensor_copy(
                            out=state["stage_t"][:, (b % SG) * 128:
                                                 (b % SG) * 128 + OUT],
                            in_=ph2[:])
                        nc.vector.tensor_copy(
                            out=self2_sb[g][:, b * OUT:(b + 1) * OUT],
                            in_=ph2[:])
                    else:
                        z2 = ep.tile([128, OUT], f32, tag="z2")
                        nc.vector.tensor_scalar(
                            out=z2[:], in0=ps, scalar1=dcol,
                            scalar2=None, op0=mybir.AluOpType.mult)
                        nc.vector.tensor_tensor(
                            out=state["stage_t"][:, (b % SG) * OUT:
                                                 (b % SG + 1) * OUT],
                            in0=z2[:], in1=b2_sb[g][:],
                            op=mybir.AluOpType.add)
                    state["nstage"] += 1
                    if state["nstage"] == SG or b == NBLK - 1:
                        flush()

                # zero-edge blocks are impossible at this edge density
                assert not m["zero_blocks"], m["zero_blocks"]

                for (ts, ntl, half) in groups:
                    gt = gp.tile([128, GSZ_TILES * 128], DTT, tag="gt")
                    nc.gpsimd.dma_gather(
                        out_ap=gt[:, :ntl * 128]
                            .rearrange("p (t c) -> p t c", c=128),
                        in_ap=halves[half],
                        idxs_ap=isb[:, ts * 8:(ts + ntl) * 8],
                        num_idxs=ntl * 128,
                        num_idxs_reg=ntl * 128,
                        elem_size=128,
                        single_packet=False,
                        queue_num=gq % NQ,
                    )
                    gq += 1
                    for k in range(ntl):
                        gt_of[ts + k] = (gt, k * 128)
                    for s0 in range(ts, ts + ntl, SK):
                        ssz = min(SK, ts + ntl - s0)
                        stb = stp.tile([128, SK * 128], DTT, tag="stb")
                        nc.vector.tensor_tensor(
                            out=stb[:, :ssz * 128]
                                .rearrange("p (t c) -> p t c", c=128),
                            in0=dsb[:, s0:s0 + ssz]
                                .rearrange("p (t c) -> p t c", c=1)
                                .to_broadcast([128, ssz, 128]),
                            in1=iota_sb[:].rearrange("p (t c) -> p t c", t=1)
                                .to_broadcast([128, ssz, 128]),
                            op=mybir.AluOpType.is_equal,
                        )
                        for k in range(ssz):
                            stb_of[s0 + k] = (stb, k * 128)
                    for j in range(ts, ts + ntl):
                        b, first, last = sched[j]
                        if first:
                            open_block(b)
                        gtile, goff = gt_of[j]
                        stile, soff = stb_of[j]
                        nc.tensor.matmul(
                            out=ps_of[b],
                            lhsT=stile[:, soff:soff + 128],
                            rhs=gtile[:, goff:goff + D],
                            start=False, stop=last,
                        )
                        if last:
                            epilogue(b, ps_of.pop(b))
                        if j in sg_last and chunk_cb is not None:
                            chunk_cb(sg_last[j] + 1)
                    if interleave is not None:
                        next(interleave, None)

            def chunk_fire(g):
                fired = set()

                def cb(sgs_done):
                    for k, (a, b) in enumerate(CHUNK_SGS):
                        if sgs_done >= b and k not in fired:
                            fired.add(k)
                            ag_chunk(g, k)
                return cb

            ax = stage_A_chunks("x")
            for _ in ax:
                pass
            ay = stage_A_chunks("y")
            for _ in ay:
                pass
            edge_pass("x", 1, chunk_cb=chunk_fire("x"))
            edge_pass("y", 1, chunk_cb=chunk_fire("y"))
            edge_pass("x", 2)
            edge_pass("y", 2)

            if DEBUG:
                for g in gs:
                    h2v = H2all[g].ap().rearrange("(nb p) c -> nb p c", p=128)
                    dbv = DBGA[g].ap().rearrange("(nb p) c -> nb p c", p=128)
                    for t0 in range(0, TBL2 // 128, 16):
                        csz = min(16, TBL2 // 128 - t0)
                        dv = h0p.tile([128, 16 * 128], DTT, tag="dbg")
                        nc.sync.dma_start(
                            out=dv[:, :csz * 128]
                                .rearrange("p (nb c) -> p nb c", c=128),
                            in_=h2v[t0:t0 + csz].rearrange("nb p c -> p nb c"))
                        nc.sync.dma_start(
                            out=dbv[t0:t0 + csz].rearrange("nb p c -> p nb c"),
                            in_=dv[:, :csz * 128]
                                .rearrange("p (nb c) -> p nb c", c=128))

    nc.compile()
    return nc


def _in_maps(meta, inputs):
    np_t = ml_dtypes.bfloat16
    iota = np.broadcast_to(np.arange(128, dtype=np.float32), (128, 128)).copy()
    shared = {"iota": iota}
    weights = {
        "x": (inputs["W1x"], inputs["b1x"], inputs["W2x"], inputs["b2x"],
              inputs["x_data_matrix"]),
        "y": (inputs["W1y"], inputs["b1y"], inputs["W2y"], inputs["b2y"],
              inputs["y_data_matrix"]),
    }
    xts = {}
    for g in ("x", "y"):
        w1, b1, w2, b2, xd = weights[g]
        dis = meta[g]["dis"]
        xt = np.zeros((128, NPAD), np.float32)
        xt[:, :N] = (np.asarray(xd, np.float32) * dis[:, None]).T
        xts[g] = xt.astype(np_t)
        shared[f"xt_{g}"] = xts[g]
        shared[f"w1_{g}"] = np.asarray(w1, np.float32).astype(np_t)
        shared[f"w2_{g}"] = np.asarray(w2, np.float32).astype(np_t)
        shared[f"b1_{g}"] = np.broadcast_to(
            np.asarray(b1, np.float32), (128, HID)).copy()
        shared[f"b2_{g}"] = np.broadcast_to(
            np.asarray(b2, np.float32), (128, OUT)).copy()
    maps = []
    for c in range(NC):
        m = dict(shared)
        for g in ("x", "y"):
            mg = meta[g]
            m[f"idx1_{g}"] = mg["l1"]["idx"][c]
            m[f"idx2_{g}"] = mg["l2"]["idx"][c]
            m[f"dstl1_{g}"] = mg["l1"]["dstl"][c]
            m[f"dstl2_{g}"] = mg["l2"]["dstl"][c]
            m[f"disb_{g}"] = mg["disb"][c]
            xs = np.zeros((128, SHP), np.float32)
            lo = c * SH
            hi = min(lo + SHP, NPAD)
            xs[:, :hi - lo] = xts[g][:, lo:hi].astype(np.float32)
            xs[:, SH:] = 0.0
            m[f"xself_{g}"] = xs.astype(np_t)
        maps.append(m)
    return maps


def run(inputs, trace=False):
    meta = {
        "x": _graph_meta(inputs["x_edge_index"]),
        "y": _graph_meta(inputs["y_edge_index"]),
    }
    nc = _build(meta)
    maps = _in_maps(meta, inputs)
    kwargs = {}
    if trace:
        kwargs = dict(trace=True, trace_cores=[0])
    res = run_bass_kernel_spmd(nc, maps, core_ids=list(range(NC)), **kwargs)
    outs = {}
    for g in ("x", "y"):
        full = np.empty((N, OUT), np.float32)
        for c in range(NC):
            full[c * SH:(c + 1) * SH] = res.results[c][f"out_{g}"][:SH]
        outs[g] = full
    return (outs["x"], outs["y"]), res


def kernel(**inputs):
    (ox, oy), _ = run(inputs)
    return ox, oy
